# revision 27
# baseline (speedup 1.0000x reference)
"""Self-contained TRN2 kernel for the bidirectional attention correction.

kernel(hl, hr) -> (mu_lr, mu_rl), matching:
    hl_n = rownorm(hl); hr_n = rownorm(hr)
    a = hl_n @ hr_n.T
    mu_lr = hr_n - softmax(a, 1).T @ hl_n
    mu_rl = hl_n - softmax(a, 0) @ hr_n

Runs SPMD on 8 NeuronCores: core c owns rows [c*1024,(c+1)*1024) of hl and
hr. All three 8192x8192x1024 GEMMs run in fp8 DoubleRow. exp(a) is computed
in both orientations ([n,m] in P1, [m,n] in P1T) straight from the gathered
fp8 operands, so no PE transposes of the 8 MB exp array are needed. Column
sums come from DoubleRow ones-matmuls inside P1 so the AllReduce fires
early; the 1/s fold then runs on the otherwise-idle Vector engine during
P1T. GpSimd carries collectives plus collective-adjacent DMAs so no compute
queue ever waits on a collective.
"""

import sys

for _p in ("/opt/trn_rl_repo",):
    if _p not in sys.path:
        sys.path.insert(0, _p)

from contextlib import ExitStack

import numpy as np

import concourse.bass as bass
import concourse.tile as tile
from concourse import bacc, mybir
from concourse.masks import make_identity
from concourse.tile import add_dep_helper

F32 = mybir.dt.float32
BF16 = mybir.dt.bfloat16
FP8 = mybir.dt.float8e4

ADD = mybir.AluOpType.add
MULT = mybir.AluOpType.mult
BYPASS = mybir.AluOpType.bypass
EXP = mybir.ActivationFunctionType.Exp
COPY = mybir.ActivationFunctionType.Copy
SQUARE = mybir.ActivationFunctionType.Square
AXL_X = mybir.AxisListType.X
DROW = mybir.MatmulPerfMode.DoubleRow


def build(C=8, NL=1024, M=8192, D=1024, stop_after="full"):
    """Build + compile the SPMD Bass graph."""
    PB = NL // 128          # local row blocks (8)
    DK = D // 128           # 128-chunks over D (8)
    JB = M // 128           # j 128-blocks (64)
    W1 = 512
    NLH = NL // 2           # 512
    DW = 512
    DH = D // DW            # 2
    SP = 64.0               # fp8 scale on P1 operands
    SE = SP * SP            # a-scale in psum (4096)
    S1 = float(8 * M)       # hl' scale
    S2 = float(M // 2)      # exp_aT sinv scale
    S8 = 8.0                # hrn8 / hl8 fp8 scale
    S3 = 8192.0             # vlr fp8 scale
    groups = [list(range(C))]
    LVL = {"prep": 0, "p1": 1, "p2b": 2, "full": 3}[stop_after]

    nc = bacc.Bacc("TRN2", target_bir_lowering=False, debug=False, num_devices=C)

    hl_in = nc.dram_tensor("hl", [NL, D], F32, kind="ExternalInput").ap()
    hr_in = nc.dram_tensor("hr", [NL, D], F32, kind="ExternalInput").ap()
    mu_lr_o = nc.dram_tensor("mu_lr", [NL, D], F32, kind="ExternalOutput").ap()
    mu_rl_o = nc.dram_tensor("mu_rl", [NL, D], F32, kind="ExternalOutput").ap()

    with tile.TileContext(nc) as tc, ExitStack() as ctx:
        dram = ctx.enter_context(tc.tile_pool(name="dram", bufs=1, space="DRAM"))
        sb = ctx.enter_context(tc.tile_pool(name="sb", bufs=1))
        ps = ctx.enter_context(tc.tile_pool(name="ps", bufs=1, space="PSUM"))

        # ---- internal DRAM ----
        hln_d = dram.tile([NL, D], BF16)           # hl_n rows (P2b adds)
        hrn_d = dram.tile([NL, D], BF16)           # hr_n rows (final subs)
        hrnT_loc = [dram.tile([D, NLH], FP8, name=f"hrnT_loc{h}")
                    for h in range(2)]
        hrnT_all = [dram.tile([C, D, NLH], FP8, name=f"hrnT_all{h}",
                              addr_space="Shared") for h in range(2)]
        hrn8_loc = dram.tile([NL, D], FP8)
        hrn8_all = dram.tile([C, NL, D], FP8, addr_space="Shared")
        s_loc = dram.tile([M], F32)
        s_glob = dram.tile([M], F32, addr_space="Shared")
        vlr_h = [dram.tile([M // 2, D], FP8, name=f"vlr_h{x}")
                 for x in range(2)]
        vred_h = [dram.tile([NL // 2, D], FP8, name=f"vred_h{x}")
                  for x in range(2)]

        # ---- SBUF resident ----
        exp_a = sb.tile([128, PB, M], FP8, name="exp_a")       # exp(a) [n,m]
        exp_aT = sb.tile([128, JB, NL], FP8, name="exp_aT")    # exp(a.T)[m,n]
        hl_nT = sb.tile([128, DK, NL], FP8, name="hl_nT")      # (hl_n*SP).T
        hl8 = sb.tile([128, PB, D], FP8, name="hl8")           # hl_n*8 -> hl'
        rt = sb.tile([128, 2, DK, NL], FP8, name="rt")         # P1 rhs stream
        rb = sb.tile([128, 3, PB, DW], FP8, name="rb")         # P2b rhs stream
        ld_st = sb.tile([128, 3, D], F32, name="ld_st")
        nrm_st = sb.tile([128, 2, D], BF16, name="nrm_st")
        trT_st = sb.tile([128, 1, DK, 128], FP8, name="trT_st")
        vlr_st = sb.tile([128, 3, D], FP8, name="vlr_st")
        out_st = sb.tile([128, 2, DW], F32, name="out_st")
        hlb_st = sb.tile([128, 2, DW], BF16, name="hlb_st")
        fin_st = sb.tile([128, 2, DW], F32, name="fin_st")
        vred_st = sb.tile([128, 4, DW], FP8, name="vred_st")
        hrn_st = sb.tile([128, 2, DW], BF16, name="hrn_st")
        # consts / stats
        ident_b = sb.tile([128, 128], BF16, name="ident_b")
        ones_e = sb.tile([128, 2, 16], FP8, name="ones_e")
        stats = sb.tile([128, 384], F32, name="stats")
        r_parts = stats[:, 0:64].rearrange("p (a b) -> p a b", a=PB)
        r_red = stats[:, 64:64 + PB]
        r_red3 = stats[:, 64:64 + PB].rearrange("p (a b) -> p a b", b=1)
        rinv = stats[:, 72:72 + PB]
        s_sb = stats[:, 80:144]
        srec = stats[:, 144:208]
        sinv = stats[:, 208:272]
        nrm = stats[:, 272:304].rearrange("p (a b) -> p a b", a=16)  # [p,16,2]

        make_identity(nc, ident_b)
        nc.vector.memset(ones_e, 1.0)

        # ================= prep: normalize, transpose, gather ===============
        def norm_chunk(src, pb, it):
            ld = ld_st[:, it % 3, :]
            nc.gpsimd.dma_start(out=ld, in_=src[pb * 128:(pb + 1) * 128, :])
            nm = nrm[:, it, :]
            sq = ps.tile([128, D], F32, tag="mm", bufs=3, name=f"sq{it}")
            nc.scalar.activation(out=sq, in_=ld, func=SQUARE,
                                 accum_out=nm[:, 0:1])
            nc.scalar.sqrt(out=nm[:, 1:2], in_=nm[:, 0:1])
            nc.vector.reciprocal(out=nm[:, 0:1], in_=nm[:, 1:2])
            nst = nrm_st[:, it % 2, :]
            nc.vector.tensor_scalar_mul(out=nst, in0=ld, scalar1=nm[:, 0:1])
            return nst

        hrn8_rows = hrn8_loc.rearrange("(pb p) d -> p pb d", p=128)

        def transposes(nst, uid, evac_out):
            # 8 128x128 PE transposes, packed 4-per-psum-tile; 2 wide evacs
            for q in range(2):
                pst = ps.tile([128, 4, 128], BF16, tag="mm", bufs=3,
                              name=f"tp{uid}_{q}")
                for x in range(4):
                    dk = 4 * q + x
                    nc.tensor.transpose(pst[:, x, :],
                                        nst[:, dk * 128:(dk + 1) * 128],
                                        ident_b)
                if q == 0:
                    nc.vector.tensor_scalar_mul(out=evac_out(q), in0=pst,
                                                scalar1=SP)
                else:
                    nc.scalar.activation(out=evac_out(q), in_=pst,
                                         func=COPY, scale=SP)

        def hr_chunk(pb, it):
            nst = norm_chunk(hr_in, pb, it)
            nc.sync.dma_start(out=hrn_d[pb * 128:(pb + 1) * 128, :], in_=nst)
            t8 = vlr_st[:, 0, :]
            nc.vector.tensor_scalar_mul(out=t8, in0=nst, scalar1=S8)
            nc.sync.dma_start(out=hrn8_rows[:, pb, :], in_=t8)
            ts = trT_st[:, 0, :, :]
            transposes(nst, pb, lambda q: ts[:, 4 * q:4 * q + 4, :])
            h, pq = divmod(pb, PB // 2)
            nc.sync.dma_start(
                out=hrnT_loc[h].rearrange("(dk p) j -> p dk j", p=128)
                [:, :, pq * 128:(pq + 1) * 128],
                in_=ts)

        def hl_chunk(pb, it):
            nst = norm_chunk(hl_in, pb, it)
            nc.sync.dma_start(out=hln_d[pb * 128:(pb + 1) * 128, :], in_=nst)
            nc.vector.tensor_scalar_mul(out=hl8[:, pb, :], in0=nst,
                                        scalar1=S8)
            transposes(nst, 8 + pb,
                       lambda q: hl_nT[:, 4 * q:4 * q + 4,
                                       pb * 128:(pb + 1) * 128])

        for pb in range(PB // 2):
            hr_chunk(pb, pb)
        ag0a = nc.gpsimd.collective_compute(
            "AllGather", BYPASS, replica_groups=groups,
            ins=[hrnT_loc[0].opt()], outs=[hrnT_all[0].opt()])
        for pb in range(PB // 2, PB):
            hr_chunk(pb, pb)
        ag0b = nc.gpsimd.collective_compute(
            "AllGather", BYPASS, replica_groups=groups,
            ins=[hrnT_loc[1].opt()], outs=[hrnT_all[1].opt()])
        add_dep_helper(ag0b.ins, ag0a.ins, sync=False,
                       reason="gather halves in order")
        for pb in range(PB):
            hl_chunk(pb, PB + pb)

        last_rt = [None]

        def load_rt(b):
            for h in range(2):
                last_rt[0] = nc.gpsimd.dma_start(
                    out=rt[:, b % 2, :, h * NLH:(h + 1) * NLH],
                    in_=hrnT_all[h][b].rearrange("(dk p) j -> p dk j",
                                                 p=128))

        # ====== P1: exp(a) [n,m] + row sums (accum) + col sums (ones-mm) ====
        if LVL >= 1:
            load_rt(0)
            load_rt(1)
            for b in range(C):
                rt_t = rt[:, b % 2, :, :]
                j0 = b * NL
                for ib in range(PB):
                    pa = ps.tile([128, D], F32, tag="mm", bufs=3,
                                 name=f"pa{b}_{ib}")
                    for kp in range(DK // 2):
                        for jh in range(2):
                            nc.tensor.matmul(
                                pa[:, jh * W1:(jh + 1) * W1],
                                lhsT=hl_nT[:, 2 * kp:2 * kp + 2,
                                           ib * 128:(ib + 1) * 128],
                                rhs=rt_t[:, 2 * kp:2 * kp + 2,
                                         jh * W1:(jh + 1) * W1],
                                start=(kp == 0), stop=(kp == DK // 2 - 1),
                                perf_mode=DROW)
                    nc.scalar.activation(
                        out=exp_a[:, ib, j0:j0 + NL], in_=pa, func=EXP,
                        scale=1.0 / SE, accum_out=r_parts[:, ib, b:b + 1])
                # column sums of this j-block via DoubleRow ones-matmul
                cs = ps.tile([1, D], F32, tag="colsum", bufs=1,
                             name=f"cs{b}")
                for jh in range(2):
                    for icp in range(PB // 2):
                        nc.tensor.matmul(
                            cs[:, jh * W1:(jh + 1) * W1],
                            lhsT=ones_e[:, :, 0:1],
                            rhs=exp_a[:, 2 * icp:2 * icp + 2,
                                      j0 + jh * W1:j0 + (jh + 1) * W1],
                            start=(icp == 0), stop=(icp == PB // 2 - 1),
                            perf_mode=DROW)
                nc.vector.tensor_copy(
                    out=fin_st[0:1, :, :],
                    in_=cs.rearrange("a (b c) -> a b c", b=2))
                nc.sync.dma_start(
                    out=s_loc[j0:j0 + NL].rearrange("(a b c) -> a b c",
                                                    a=1, b=2),
                    in_=fin_st[0:1, :, :])
                if b + 2 < C:
                    load_rt(b + 2)

            # r -> rinv ; hl8 -> hl' = hl_n*rinv*S1 (in place)
            nc.vector.tensor_reduce(out=r_red3, in_=r_parts, op=ADD,
                                    axis=AXL_X)
            nc.vector.reciprocal(out=rinv, in_=r_red)
            for pb in range(PB):
                nc.vector.tensor_scalar(
                    out=hl8[:, pb, :], in0=hl8[:, pb, :],
                    scalar1=rinv[:, pb:pb + 1], scalar2=S1 / S8,
                    op0=MULT, op1=MULT)

            # s: AllReduce early; readback + sinv on gpsimd/vector.
            # P1T rt loads move to the sync queue so they never sit behind
            # a collective on gpsimd; the hrn8 gather runs after the (small)
            # AllReduce and overlaps P1T on the CC engine.
            def load_rt_sync(b):
                for h in range(2):
                    nc.sync.dma_start(
                        out=rt[:, b % 2, :, h * NLH:(h + 1) * NLH],
                        in_=hrnT_all[h][b].rearrange("(dk p) j -> p dk j",
                                                     p=128))

            load_rt_sync(0)
            load_rt_sync(1)
            nc.gpsimd.collective_compute(
                "AllReduce", ADD, replica_groups=groups,
                ins=[s_loc.opt()], outs=[s_glob.opt()])
            nc.gpsimd.dma_start(
                out=s_sb, in_=s_glob.rearrange("(b p) -> p b", p=128))
            ag2_i = nc.gpsimd.collective_compute(
                "AllGather", BYPASS, replica_groups=groups,
                ins=[hrn8_loc.opt()], outs=[hrn8_all.opt()])
            add_dep_helper(ag2_i.ins, last_rt[0].ins, sync=False,
                           reason="hrn8 gather after P1 rt loads")
            nc.vector.reciprocal(out=srec, in_=s_sb)
            nc.vector.tensor_scalar_mul(out=sinv, in0=srec, scalar1=S2)

            # ====== P1T: exp(a.T) [m,n]; fold sinv*S2 in place on Vector ====
            for b in range(C):
                rt_t = rt[:, b % 2, :, :]
                for ms in range(PB):
                    jb = b * PB + ms
                    pt = ps.tile([128, D], F32, tag="mm", bufs=3,
                                 name=f"pt{jb}")
                    for kp in range(DK // 2):
                        for k in range(2):
                            nc.tensor.matmul(
                                pt[:, k * W1:(k + 1) * W1],
                                lhsT=rt_t[:, 2 * kp:2 * kp + 2,
                                          ms * 128:(ms + 1) * 128],
                                rhs=hl_nT[:, 2 * kp:2 * kp + 2,
                                          k * W1:(k + 1) * W1],
                                start=(kp == 0), stop=(kp == DK // 2 - 1),
                                perf_mode=DROW)
                    nc.scalar.activation(
                        out=exp_aT[:, jb, :], in_=pt, func=EXP,
                        scale=1.0 / SE)
                    nc.vector.tensor_scalar_mul(
                        out=exp_aT[:, jb, :], in0=exp_aT[:, jb, :],
                        scalar1=sinv[:, jb:jb + 1])
                if b + 2 < C:
                    load_rt_sync(b + 2)

        # ====== P2b: mu_rl = hl_n - exp_aT_scaled.T @ hrn8 ==================
        # Runs BEFORE P2a so its rhs streaming never overlaps a
        # ReduceScatter (DMAs issued during an RS stall for its duration).
        if LVL >= 2:
            for dh in range(DH):
                for ih in range(2):
                    acc2 = [ps.tile([128, D], F32, tag="mm", bufs=3,
                                    name=f"acc{dh}_{ih}_{g}")
                            for g in range(2)]
                    for bb in range(C):
                        rbt = rb[:, bb % 3, :, :]
                        eng = nc.sync if bb % 2 == 0 else nc.scalar
                        eng.dma_start(
                            out=rbt,
                            in_=hrn8_all[bb].rearrange(
                                "(jb p) d -> p jb d", p=128)
                            [:, :, dh * DW:(dh + 1) * DW])
                        for l_ in range(4):
                            jbp = bb * 4 + l_
                            for xi in range(4):
                                ib = 4 * ih + xi
                                nc.tensor.matmul(
                                    acc2[xi // 2][:, (xi % 2) * DW:
                                                  (xi % 2 + 1) * DW],
                                    lhsT=exp_aT[:, 2 * jbp:2 * jbp + 2,
                                                ib * 128:(ib + 1) * 128],
                                    rhs=rbt[:, 2 * l_:2 * l_ + 2, :],
                                    start=(jbp == 0),
                                    stop=(jbp == JB // 2 - 1),
                                    perf_mode=DROW)
                    for xi in range(4):
                        ib = 4 * ih + xi
                        st = out_st[:, xi % 2, :]
                        hb = hlb_st[:, xi % 2, :]
                        nc.sync.dma_start(
                            out=hb, in_=hln_d[ib * 128:(ib + 1) * 128,
                                              dh * DW:(dh + 1) * DW])
                        nc.vector.tensor_scalar_mul(
                            out=st, in0=acc2[xi // 2][:, (xi % 2) * DW:
                                                      (xi % 2 + 1) * DW],
                            scalar1=-1.0 / (S2 * S8))
                        nc.vector.tensor_add(out=st, in0=st, in1=hb)
                        nc.scalar.dma_start(
                            out=mu_rl_o[ib * 128:(ib + 1) * 128,
                                        dh * DW:(dh + 1) * DW], in_=st)

        # ====== P2a: vlr = exp_a.T @ hl'  (row-permuted halves, 2x RS) ======
        # No input DMA at all (both operands SBUF-resident), so the two
        # ReduceScatters only ever contend with its output writes.
        def vlr_row(jb):
            c0, lb = divmod(jb, PB)
            return (0, c0 * 512 + lb * 128) if lb < 4 else \
                   (1, c0 * 512 + (lb - 4) * 128)

        last_v = [None, None]
        if LVL >= 3:
            a_jbs = [jb for jb in range(JB) if jb % PB < 4]
            b_jbs = [jb for jb in range(JB) if jb % PB >= 4]
            for half, jbs in enumerate((a_jbs, b_jbs)):
                for idx, jb in enumerate(jbs):
                    pl = ps.tile([128, D], F32, tag="mm", bufs=3,
                                 name=f"pl{jb}")
                    for icp in range(PB // 2):
                        for dh in range(DH):
                            nc.tensor.matmul(
                                pl[:, dh * DW:(dh + 1) * DW],
                                lhsT=exp_a[:, 2 * icp:2 * icp + 2,
                                           jb * 128:(jb + 1) * 128],
                                rhs=hl8[:, 2 * icp:2 * icp + 2,
                                        dh * DW:(dh + 1) * DW],
                                start=(icp == 0), stop=(icp == PB // 2 - 1),
                                perf_mode=DROW)
                    vst = vlr_st[:, idx % 3, :]
                    last_v[half] = nc.vector.tensor_scalar_mul(
                        out=vst, in0=pl, scalar1=S3 / S1)
                    _, r0 = vlr_row(jb)
                    eng = nc.sync if idx % 2 == 0 else nc.scalar
                    eng.dma_start(out=vlr_h[half][r0:r0 + 128, :], in_=vst)
                nc.gpsimd.collective_compute(
                    "ReduceScatter", ADD, replica_groups=groups,
                    ins=[vlr_h[half].opt()], outs=[vred_h[half].opt()])

        # ================= final: mu_lr = hr_n - vred =======================
        if LVL >= 3:
            for pb in range(PB):
                half, r0 = (0, pb * 128) if pb < 4 else (1, (pb - 4) * 128)
                ld_eng = nc.scalar if half == 0 else nc.gpsimd
                for dh in range(DH):
                    it = pb * DH + dh
                    vs = vred_st[:, it % 4, :]
                    ld_eng.dma_start(
                        out=vs, in_=vred_h[half][r0:r0 + 128,
                                                 dh * DW:(dh + 1) * DW])
                    hs = hrn_st[:, it % 2, :]
                    nc.sync.dma_start(
                        out=hs, in_=hrn_d[pb * 128:(pb + 1) * 128,
                                          dh * DW:(dh + 1) * DW])
                    vx = hlb_st[:, it % 2, :]
                    nc.scalar.activation(out=vx, in_=vs, func=COPY,
                                         scale=1.0 / S3)
                    st = fin_st[:, it % 2, :]
                    sub_i = nc.vector.tensor_sub(out=st, in0=hs, in1=vx)
                    if last_v[half] is not None:
                        add_dep_helper(sub_i.ins, last_v[half].ins,
                                       sync=False, reason="after P2a evacs")
                    nc.scalar.dma_start(
                        out=mu_lr_o[pb * 128:(pb + 1) * 128,
                                    dh * DW:(dh + 1) * DW], in_=st)

        # dummy writes for any output a stopped-early build didn't produce
        if LVL < 2:
            for pb in range(PB):
                for dh in range(DH):
                    st = out_st[:, pb % 2, :]
                    nc.vector.memset(st, 0.0)
                    nc.sync.dma_start(
                        out=mu_rl_o[pb * 128:(pb + 1) * 128,
                                    dh * DW:(dh + 1) * DW], in_=st)
        if LVL < 3:
            for pb in range(PB):
                for dh in range(DH):
                    st = fin_st[:, pb % 2, :]
                    nc.vector.memset(st, 0.0)
                    nc.sync.dma_start(
                        out=mu_lr_o[pb * 128:(pb + 1) * 128,
                                    dh * DW:(dh + 1) * DW], in_=st)

    nc.compile()
    return nc


_NC_CACHE = {}


def _get_nc():
    if "nc" not in _NC_CACHE:
        _NC_CACHE["nc"] = build(C=8, NL=1024, M=8192, D=1024)
    return _NC_CACHE["nc"]


def kernel(hl, hr):
    """Full inputs in, full outputs out; distributes across 8 cores."""
    from concourse.bass_utils import run_bass_kernel_spmd

    C, NL = 8, 1024
    hl = np.ascontiguousarray(np.asarray(hl, dtype=np.float32))
    hr = np.ascontiguousarray(np.asarray(hr, dtype=np.float32))
    nc = _get_nc()
    in_maps = [
        {"hl": np.ascontiguousarray(hl[c * NL:(c + 1) * NL]),
         "hr": np.ascontiguousarray(hr[c * NL:(c + 1) * NL])}
        for c in range(C)
    ]
    res = run_bass_kernel_spmd(nc, in_maps, list(range(C)))
    mu_lr = np.concatenate([res.results[c]["mu_lr"] for c in range(C)])
    mu_rl = np.concatenate([res.results[c]["mu_rl"] for c in range(C)])
    return mu_lr, mu_rl
